# revision 8
# baseline (speedup 1.0000x reference)
"""AngularLoss Trainium2 kernel (8 NeuronCores, SPMD data-parallel).

Computation (reference):
    t2  = tan(alpha_deg * pi/180)^2
    apn = rowsum((a + p) * n)          # [N,1]
    ap  = rowsum(a * p)                # [N,1]
    f   = 4*t2*apn - 2*(1+t2)*ap       # [N,1]
    out = logsumexp(f, axis=0)         # [1]

Strategy: shard N=262144 rows across 8 cores (32768 rows each).  Each core
streams its 3x16MB f32 shard through SBUF (SWDGE DMA casts to bf16 in the
datapath; the per-core DMA path saturates at ~384 GB/s, so the kernel is
memory-roofline-bound and everything else must hide under the stream).
Per-row dots run on DVE in bf16 TT 2x mode (plain tensor_tensor only — STT
gets no 2x).  The two products (q*n and a*p) land in one combined tile so a
single triple-fold chain + one (1x-rate) reduce serves both, writing the
per-partition (apn, ap) accumulator columns through a strided view.  Rows
accumulate in groups; each finished group's logsumexp (combine, max,
exp-accum, out-DMA) overlaps the still-streaming chunks, and the final
group is tiny so the post-stream tail is short.  Output per core is
[128, 2*G] = (m_g, s_g) per partition per group.  Host combines the
partials into the final logsumexp — no on-chip collective needed.
"""

import numpy as np

import concourse.bacc as bacc
import concourse.bass as bass
import concourse.tile as tile
from concourse import mybir
from concourse.bass_utils import run_bass_kernel_spmd

N, D = 262144, 128
NCORES = 8
N_LOCAL = N // NCORES            # 32768 rows per core
P = 128                          # partitions
ROWS_PER_PART = N_LOCAL // P     # 256 rows owned by each partition
F32 = mybir.dt.float32
BF16 = mybir.dt.bfloat16
CDT = BF16                       # on-chip compute dtype (DMA casts f32->bf16)

# row-chunk schedule (rows per partition per DMA load): 32-row (16KB DRAM
# descriptor) loads steady-state, small last chunks so the final chunk's
# compute tail is short.
CHUNKS = [32] * 7 + [8] * 4
assert sum(CHUNKS) == ROWS_PER_PART
# logsumexp column groups (each finishes as soon as its columns accumulate;
# the late groups are tiny so the post-stream tail is short, and the big
# groups close mid-stream).  Boundaries must align with chunk boundaries.
GROUPS = [96, 128, 8, 8, 8, 8]
assert sum(GROUPS) == ROWS_PER_PART
N_GROUPS = len(GROUPS)
GROUP_START = [sum(GROUPS[:g]) for g in range(N_GROUPS)]


def _group_of(col):
    g = 0
    while col >= GROUP_START[g] + GROUPS[g]:
        g += 1
    return g


def _build(c1: float, c2: float) -> bass.Bass:
    nc = bacc.Bacc()
    a_ext = nc.declare_dram_parameter("anchor", [N_LOCAL, D], F32, isOutput=False)
    p_ext = nc.declare_dram_parameter("positive", [N_LOCAL, D], F32, isOutput=False)
    n_ext = nc.declare_dram_parameter("negative", [N_LOCAL, D], F32, isOutput=False)
    out_ext = nc.declare_dram_parameter("out", [P, 2 * N_GROUPS], F32, isOutput=True)

    # Partition p owns rows [p*256, (p+1)*256): contiguous chunk per
    # partition, partition stride 128KB.
    a_v = a_ext.rearrange("(p r) d -> p r d", p=P)
    p_v = p_ext.rearrange("(p r) d -> p r d", p=P)
    n_v = n_ext.rearrange("(p r) d -> p r d", p=P)

    with tile.TileContext(nc) as tc:
        with (
            tc.tile_pool(name="ina", bufs=3) as pool_a,
            tc.tile_pool(name="inp", bufs=3) as pool_p,
            tc.tile_pool(name="inn", bufs=3) as pool_n,
            tc.tile_pool(name="q", bufs=2) as pool_q,
            tc.tile_pool(name="z", bufs=2) as pool_z,
            tc.tile_pool(name="fold", bufs=2) as pool_f1,
            tc.tile_pool(name="fold2", bufs=2) as pool_f2,
            tc.tile_pool(name="fold3", bufs=2) as pool_f3,
            tc.tile_pool(name="acc", bufs=1) as pool_acc,
        ):
            # Per-group accumulators [P, 2, G]: row 0 = apn, row 1 = ap.
            acc_g = [
                pool_acc.tile([P, 2 * GROUPS[g]], F32, name=f"acc{g}", tag=f"acc{g}")
                for g in range(N_GROUPS)
            ]
            f_g = [
                pool_acc.tile([P, GROUPS[g]], F32, name=f"f{g}", tag=f"f{g}")
                for g in range(N_GROUPS)
            ]
            tmp_g = [
                pool_acc.tile([P, GROUPS[g]], F32, name=f"tmp{g}", tag=f"tmp{g}")
                for g in range(N_GROUPS)
            ]
            expf_g = [
                pool_acc.tile([P, GROUPS[g]], F32, name=f"expf{g}", tag=f"expf{g}")
                for g in range(N_GROUPS)
            ]
            negm_g = [
                pool_acc.tile([P, 1], F32, name=f"negm{g}", tag=f"negm{g}")
                for g in range(N_GROUPS)
            ]
            ms_g = [
                pool_acc.tile([P, 2], F32, name=f"ms{g}", tag=f"ms{g}")
                for g in range(N_GROUPS)
            ]

            def logsumexp_group(g):
                # f = c1*apn + c2*ap, then max / exp-accum, ship [P,2] out.
                G = GROUPS[g]
                apn = acc_g[g][:, 0:G]
                ap = acc_g[g][:, G : 2 * G]
                nc.vector.tensor_scalar_mul(tmp_g[g][:], ap, c2)
                nc.vector.scalar_tensor_tensor(
                    f_g[g][:], apn, c1, tmp_g[g][:],
                    op0=mybir.AluOpType.mult, op1=mybir.AluOpType.add,
                )
                nc.vector.tensor_reduce(
                    out=ms_g[g][:, 0:1], in_=f_g[g][:],
                    axis=mybir.AxisListType.X, op=mybir.AluOpType.max,
                )
                nc.vector.tensor_scalar_mul(negm_g[g][:], ms_g[g][:, 0:1], -1.0)
                nc.scalar.activation(
                    out=expf_g[g][:], in_=f_g[g][:],
                    func=mybir.ActivationFunctionType.Exp,
                    bias=negm_g[g][:], scale=1.0,
                    accum_out=ms_g[g][:, 1:2],
                )
                nc.sync.dma_start(
                    out=out_ext[:, 2 * g : 2 * g + 2], in_=ms_g[g][:]
                )

            col = 0
            group_done = 0
            for B in CHUNKS:
                ta = pool_a.tile([P, B * D], CDT, tag="ina")
                tp = pool_p.tile([P, B * D], CDT, tag="inp")
                tn = pool_n.tile([P, B * D], CDT, tag="inn")
                # SWDGE (gpsimd) DMA casts f32 -> bf16 in the datapath.
                nc.gpsimd.dma_start(out=ta[:], in_=a_v[:, col : col + B, :])
                nc.gpsimd.dma_start(out=tp[:], in_=p_v[:, col : col + B, :])
                nc.gpsimd.dma_start(out=tn[:], in_=n_v[:, col : col + B, :])

                # q = a+p, then the two products into one combined tile:
                # z[:, :B*D] = q*n (-> apn), z[:, B*D:] = a*p (-> ap).
                tq = pool_q.tile([P, B * D], CDT, tag="q")
                tz = pool_z.tile([P, 2 * B * D], CDT, tag="z")
                nc.vector.tensor_tensor(tq[:], ta[:], tp[:], mybir.AluOpType.add)
                nc.vector.tensor_tensor(
                    tz[:, : B * D], tq[:], tn[:], mybir.AluOpType.mult
                )
                nc.vector.tensor_tensor(
                    tz[:, B * D :], ta[:], tp[:], mybir.AluOpType.mult
                )

                # Triple fold (bf16 TT 2x) then one 1x-rate reduce for both
                # halves; rows land as [2, B] via a strided output view.
                R = 2 * B
                v0 = tz[:].rearrange("p (r d) -> p r d", d=D)
                t1 = pool_f1.tile([P, R * (D // 2)], CDT, tag="fold")
                nc.vector.tensor_tensor(
                    t1[:], v0[:, :, : D // 2], v0[:, :, D // 2 :],
                    mybir.AluOpType.add,
                )
                v1 = t1[:].rearrange("p (r d) -> p r d", d=D // 2)
                t2 = pool_f2.tile([P, R * (D // 4)], CDT, tag="fold2")
                nc.vector.tensor_tensor(
                    t2[:], v1[:, :, : D // 4], v1[:, :, D // 4 :],
                    mybir.AluOpType.add,
                )
                v2 = t2[:].rearrange("p (r d) -> p r d", d=D // 4)
                t3 = pool_f3.tile([P, R * (D // 8)], CDT, tag="fold3")
                nc.vector.tensor_tensor(
                    t3[:], v2[:, :, : D // 8], v2[:, :, D // 8 :],
                    mybir.AluOpType.add,
                )

                g = _group_of(col)
                gcol = col - GROUP_START[g]
                assert gcol + B <= GROUPS[g]
                G = GROUPS[g]
                dst = acc_g[g][:].rearrange("p (h c) -> p h c", h=2)[
                    :, :, gcol : gcol + B
                ]
                nc.vector.tensor_reduce(
                    out=dst,
                    in_=t3[:].rearrange("p (r d) -> p r d", d=D // 8),
                    axis=mybir.AxisListType.X,
                    op=mybir.AluOpType.add,
                )
                col += B
                # close out any fully-accumulated logsumexp group
                while group_done < N_GROUPS and col >= GROUP_START[group_done] + GROUPS[group_done]:
                    logsumexp_group(group_done)
                    group_done += 1
    nc.compile()
    return nc


def kernel(anchor, positive, negative, alpha):
    anchor = np.ascontiguousarray(np.asarray(anchor, dtype=np.float32))
    positive = np.ascontiguousarray(np.asarray(positive, dtype=np.float32))
    negative = np.ascontiguousarray(np.asarray(negative, dtype=np.float32))
    a_rad = 2.0 * np.pi * float(np.asarray(alpha)) / 360.0
    t2 = float(np.tan(a_rad) ** 2)
    c1 = 4.0 * t2
    c2 = -2.0 * (1.0 + t2)

    nc = _build(c1, c2)
    in_maps = []
    for i in range(NCORES):
        sl = slice(i * N_LOCAL, (i + 1) * N_LOCAL)
        in_maps.append(
            {"anchor": anchor[sl], "positive": positive[sl], "negative": negative[sl]}
        )
    res = run_bass_kernel_spmd(nc, in_maps, core_ids=list(range(NCORES)))

    ms = np.concatenate([np.asarray(r["out"]) for r in res.results], axis=0)
    m = ms[:, 0::2].reshape(-1).astype(np.float64)
    s = ms[:, 1::2].reshape(-1).astype(np.float64)
    M = m.max()
    S = np.sum(s * np.exp(m - M))
    return np.array([np.log(S) + M], dtype=np.float32)


if __name__ == "__main__":
    rng = np.random.default_rng(0)
    out = kernel(
        anchor=rng.standard_normal((N, D), dtype=np.float32),
        positive=rng.standard_normal((N, D), dtype=np.float32),
        negative=rng.standard_normal((N, D), dtype=np.float32),
        alpha=np.int64(45),
    )
    print("kernel out:", out)


# revision 9
# speedup vs baseline: 1.0058x; 1.0058x over previous
"""AngularLoss Trainium2 kernel (8 NeuronCores, SPMD data-parallel).

Computation (reference):
    t2  = tan(alpha_deg * pi/180)^2
    apn = rowsum((a + p) * n)          # [N,1]
    ap  = rowsum(a * p)                # [N,1]
    f   = 4*t2*apn - 2*(1+t2)*ap       # [N,1]
    out = logsumexp(f, axis=0)         # [1]

Strategy: shard N=262144 rows across 8 cores (32768 rows each).  Each core
streams its 3x16MB f32 shard through SBUF (SWDGE DMA casts to bf16 in the
datapath; the per-core DMA path saturates at ~384 GB/s, so the kernel is
memory-roofline-bound and everything else must hide under the stream).
Per-row dots run on DVE in bf16 TT 2x mode (plain tensor_tensor only — STT
gets no 2x).  The two products (q*n and a*p) land in one combined tile so a
single triple-fold chain + one (1x-rate) reduce serves both, writing the
per-partition (apn, ap) accumulator columns through a strided view.  Rows
accumulate in groups; each finished group's logsumexp (combine, max,
exp-accum, out-DMA) overlaps the still-streaming chunks, and the final
group is tiny so the post-stream tail is short.  Output per core is
[128, 2*G] = (m_g, s_g) per partition per group.  Host combines the
partials into the final logsumexp — no on-chip collective needed.
"""

import numpy as np

import concourse.bacc as bacc
import concourse.bass as bass
import concourse.tile as tile
from concourse import mybir
from concourse.bass_utils import run_bass_kernel_spmd

N, D = 262144, 128
NCORES = 8
N_LOCAL = N // NCORES            # 32768 rows per core
P = 128                          # partitions
ROWS_PER_PART = N_LOCAL // P     # 256 rows owned by each partition
F32 = mybir.dt.float32
BF16 = mybir.dt.bfloat16
CDT = BF16                       # on-chip compute dtype (DMA casts f32->bf16)

# row-chunk schedule (rows per partition per DMA load): 32-row (16KB DRAM
# descriptor) loads steady-state, small last chunks so the final chunk's
# compute tail is short.
CHUNKS = [32] * 5 + [16] * 5 + [8] * 2
assert sum(CHUNKS) == ROWS_PER_PART
# logsumexp column groups (each finishes as soon as its columns accumulate;
# the late groups are tiny so the post-stream tail is short, and the big
# groups close mid-stream).  Boundaries must align with chunk boundaries.
GROUPS = [96, 112, 32, 8, 8]
assert sum(GROUPS) == ROWS_PER_PART
N_GROUPS = len(GROUPS)
GROUP_START = [sum(GROUPS[:g]) for g in range(N_GROUPS)]


def _group_of(col):
    g = 0
    while col >= GROUP_START[g] + GROUPS[g]:
        g += 1
    return g


def _build(c1: float, c2: float) -> bass.Bass:
    nc = bacc.Bacc()
    a_ext = nc.declare_dram_parameter("anchor", [N_LOCAL, D], F32, isOutput=False)
    p_ext = nc.declare_dram_parameter("positive", [N_LOCAL, D], F32, isOutput=False)
    n_ext = nc.declare_dram_parameter("negative", [N_LOCAL, D], F32, isOutput=False)
    out_ext = nc.declare_dram_parameter("out", [P, 2 * N_GROUPS], F32, isOutput=True)

    # Partition p owns rows [p*256, (p+1)*256): contiguous chunk per
    # partition, partition stride 128KB.
    a_v = a_ext.rearrange("(p r) d -> p r d", p=P)
    p_v = p_ext.rearrange("(p r) d -> p r d", p=P)
    n_v = n_ext.rearrange("(p r) d -> p r d", p=P)

    with tile.TileContext(nc) as tc:
        with (
            tc.tile_pool(name="ina", bufs=3) as pool_a,
            tc.tile_pool(name="inp", bufs=3) as pool_p,
            tc.tile_pool(name="inn", bufs=3) as pool_n,
            tc.tile_pool(name="q", bufs=2) as pool_q,
            tc.tile_pool(name="z", bufs=2) as pool_z,
            tc.tile_pool(name="fold", bufs=2) as pool_f1,
            tc.tile_pool(name="fold2", bufs=2) as pool_f2,
            tc.tile_pool(name="fold3", bufs=2) as pool_f3,
            tc.tile_pool(name="acc", bufs=1) as pool_acc,
        ):
            # Per-group accumulators [P, 2, G]: row 0 = apn, row 1 = ap.
            acc_g = [
                pool_acc.tile([P, 2 * GROUPS[g]], F32, name=f"acc{g}", tag=f"acc{g}")
                for g in range(N_GROUPS)
            ]
            f_g = [
                pool_acc.tile([P, GROUPS[g]], F32, name=f"f{g}", tag=f"f{g}")
                for g in range(N_GROUPS)
            ]
            tmp_g = [
                pool_acc.tile([P, GROUPS[g]], F32, name=f"tmp{g}", tag=f"tmp{g}")
                for g in range(N_GROUPS)
            ]
            expf_g = [
                pool_acc.tile([P, GROUPS[g]], F32, name=f"expf{g}", tag=f"expf{g}")
                for g in range(N_GROUPS)
            ]
            negm_g = [
                pool_acc.tile([P, 1], F32, name=f"negm{g}", tag=f"negm{g}")
                for g in range(N_GROUPS)
            ]
            ms_g = [
                pool_acc.tile([P, 2], F32, name=f"ms{g}", tag=f"ms{g}")
                for g in range(N_GROUPS)
            ]

            def logsumexp_group(g):
                # f = c1*apn + c2*ap, then max / exp-accum, ship [P,2] out.
                G = GROUPS[g]
                apn = acc_g[g][:, 0:G]
                ap = acc_g[g][:, G : 2 * G]
                nc.vector.tensor_scalar_mul(tmp_g[g][:], ap, c2)
                nc.vector.scalar_tensor_tensor(
                    f_g[g][:], apn, c1, tmp_g[g][:],
                    op0=mybir.AluOpType.mult, op1=mybir.AluOpType.add,
                )
                nc.vector.tensor_reduce(
                    out=ms_g[g][:, 0:1], in_=f_g[g][:],
                    axis=mybir.AxisListType.X, op=mybir.AluOpType.max,
                )
                nc.vector.tensor_scalar_mul(negm_g[g][:], ms_g[g][:, 0:1], -1.0)
                nc.scalar.activation(
                    out=expf_g[g][:], in_=f_g[g][:],
                    func=mybir.ActivationFunctionType.Exp,
                    bias=negm_g[g][:], scale=1.0,
                    accum_out=ms_g[g][:, 1:2],
                )
                nc.sync.dma_start(
                    out=out_ext[:, 2 * g : 2 * g + 2], in_=ms_g[g][:]
                )

            col = 0
            group_done = 0
            for B in CHUNKS:
                ta = pool_a.tile([P, B * D], CDT, tag="ina")
                tp = pool_p.tile([P, B * D], CDT, tag="inp")
                tn = pool_n.tile([P, B * D], CDT, tag="inn")
                # SWDGE (gpsimd) DMA casts f32 -> bf16 in the datapath.
                nc.gpsimd.dma_start(out=ta[:], in_=a_v[:, col : col + B, :])
                nc.gpsimd.dma_start(out=tp[:], in_=p_v[:, col : col + B, :])
                nc.gpsimd.dma_start(out=tn[:], in_=n_v[:, col : col + B, :])

                # q = a+p, then the two products into one combined tile:
                # z[:, :B*D] = q*n (-> apn), z[:, B*D:] = a*p (-> ap).
                tq = pool_q.tile([P, B * D], CDT, tag="q")
                tz = pool_z.tile([P, 2 * B * D], CDT, tag="z")
                nc.vector.tensor_tensor(tq[:], ta[:], tp[:], mybir.AluOpType.add)
                nc.vector.tensor_tensor(
                    tz[:, : B * D], tq[:], tn[:], mybir.AluOpType.mult
                )
                nc.vector.tensor_tensor(
                    tz[:, B * D :], ta[:], tp[:], mybir.AluOpType.mult
                )

                # Triple fold (bf16 TT 2x) then one 1x-rate reduce for both
                # halves; rows land as [2, B] via a strided output view.
                R = 2 * B
                v0 = tz[:].rearrange("p (r d) -> p r d", d=D)
                t1 = pool_f1.tile([P, R * (D // 2)], CDT, tag="fold")
                nc.vector.tensor_tensor(
                    t1[:], v0[:, :, : D // 2], v0[:, :, D // 2 :],
                    mybir.AluOpType.add,
                )
                v1 = t1[:].rearrange("p (r d) -> p r d", d=D // 2)
                t2 = pool_f2.tile([P, R * (D // 4)], CDT, tag="fold2")
                nc.vector.tensor_tensor(
                    t2[:], v1[:, :, : D // 4], v1[:, :, D // 4 :],
                    mybir.AluOpType.add,
                )
                v2 = t2[:].rearrange("p (r d) -> p r d", d=D // 4)
                t3 = pool_f3.tile([P, R * (D // 8)], CDT, tag="fold3")
                nc.vector.tensor_tensor(
                    t3[:], v2[:, :, : D // 8], v2[:, :, D // 8 :],
                    mybir.AluOpType.add,
                )

                g = _group_of(col)
                gcol = col - GROUP_START[g]
                assert gcol + B <= GROUPS[g]
                G = GROUPS[g]
                dst = acc_g[g][:].rearrange("p (h c) -> p h c", h=2)[
                    :, :, gcol : gcol + B
                ]
                nc.vector.tensor_reduce(
                    out=dst,
                    in_=t3[:].rearrange("p (r d) -> p r d", d=D // 8),
                    axis=mybir.AxisListType.X,
                    op=mybir.AluOpType.add,
                )
                col += B
                # close out any fully-accumulated logsumexp group
                while group_done < N_GROUPS and col >= GROUP_START[group_done] + GROUPS[group_done]:
                    logsumexp_group(group_done)
                    group_done += 1
    nc.compile()
    return nc


def kernel(anchor, positive, negative, alpha):
    anchor = np.ascontiguousarray(np.asarray(anchor, dtype=np.float32))
    positive = np.ascontiguousarray(np.asarray(positive, dtype=np.float32))
    negative = np.ascontiguousarray(np.asarray(negative, dtype=np.float32))
    a_rad = 2.0 * np.pi * float(np.asarray(alpha)) / 360.0
    t2 = float(np.tan(a_rad) ** 2)
    c1 = 4.0 * t2
    c2 = -2.0 * (1.0 + t2)

    nc = _build(c1, c2)
    in_maps = []
    for i in range(NCORES):
        sl = slice(i * N_LOCAL, (i + 1) * N_LOCAL)
        in_maps.append(
            {"anchor": anchor[sl], "positive": positive[sl], "negative": negative[sl]}
        )
    res = run_bass_kernel_spmd(nc, in_maps, core_ids=list(range(NCORES)))

    ms = np.concatenate([np.asarray(r["out"]) for r in res.results], axis=0)
    m = ms[:, 0::2].reshape(-1).astype(np.float64)
    s = ms[:, 1::2].reshape(-1).astype(np.float64)
    M = m.max()
    S = np.sum(s * np.exp(m - M))
    return np.array([np.log(S) + M], dtype=np.float32)


if __name__ == "__main__":
    rng = np.random.default_rng(0)
    out = kernel(
        anchor=rng.standard_normal((N, D), dtype=np.float32),
        positive=rng.standard_normal((N, D), dtype=np.float32),
        negative=rng.standard_normal((N, D), dtype=np.float32),
        alpha=np.int64(45),
    )
    print("kernel out:", out)


# revision 10
# speedup vs baseline: 1.0139x; 1.0081x over previous
"""AngularLoss Trainium2 kernel (8 NeuronCores, SPMD data-parallel).

Computation (reference):
    t2  = tan(alpha_deg * pi/180)^2
    apn = rowsum((a + p) * n)          # [N,1]
    ap  = rowsum(a * p)                # [N,1]
    f   = 4*t2*apn - 2*(1+t2)*ap       # [N,1]
    out = logsumexp(f, axis=0)         # [1]

Strategy: shard N=262144 rows across 8 cores (32768 rows each).  Each core
streams its 3x16MB f32 shard through SBUF (SWDGE DMA casts to bf16 in the
datapath; the per-core DMA path saturates at ~384 GB/s, so the kernel is
memory-roofline-bound and everything else must hide under the stream).
Per-row dots run on DVE in bf16 TT 2x mode (plain tensor_tensor only — STT
gets no 2x).  The two products (q*n and a*p) land in one combined tile so a
single triple-fold chain + one (1x-rate) reduce serves both, writing the
per-partition (apn, ap) accumulator columns through a strided view.  Rows
accumulate in groups; each finished group's logsumexp (combine, max,
exp-accum, out-DMA) overlaps the still-streaming chunks, and the final
group is tiny so the post-stream tail is short.  Output per core is
[128, 2*G] = (m_g, s_g) per partition per group.  Host combines the
partials into the final logsumexp — no on-chip collective needed.
"""

import numpy as np

import concourse.bacc as bacc
import concourse.bass as bass
import concourse.tile as tile
from concourse import mybir
from concourse.bass_utils import run_bass_kernel_spmd

N, D = 262144, 128
NCORES = 8
N_LOCAL = N // NCORES            # 32768 rows per core
P = 128                          # partitions
ROWS_PER_PART = N_LOCAL // P     # 256 rows owned by each partition
F32 = mybir.dt.float32
BF16 = mybir.dt.bfloat16
CDT = BF16                       # on-chip compute dtype (DMA casts f32->bf16)

# row-chunk schedule (rows per partition per DMA load): 32-row (16KB DRAM
# descriptor) loads steady-state, small last chunks so the final chunk's
# compute tail is short.
CHUNKS = [32] * 4 + [16] * 7 + [8] * 2
assert sum(CHUNKS) == ROWS_PER_PART
# logsumexp column groups (each finishes as soon as its columns accumulate;
# the late groups are tiny so the post-stream tail is short, and the big
# groups close mid-stream).  Boundaries must align with chunk boundaries.
GROUPS = [96, 112, 32, 8, 8]
assert sum(GROUPS) == ROWS_PER_PART
N_GROUPS = len(GROUPS)
GROUP_START = [sum(GROUPS[:g]) for g in range(N_GROUPS)]


def _group_of(col):
    g = 0
    while col >= GROUP_START[g] + GROUPS[g]:
        g += 1
    return g


def _build(c1: float, c2: float) -> bass.Bass:
    nc = bacc.Bacc()
    a_ext = nc.declare_dram_parameter("anchor", [N_LOCAL, D], F32, isOutput=False)
    p_ext = nc.declare_dram_parameter("positive", [N_LOCAL, D], F32, isOutput=False)
    n_ext = nc.declare_dram_parameter("negative", [N_LOCAL, D], F32, isOutput=False)
    out_ext = nc.declare_dram_parameter("out", [P, 2 * N_GROUPS], F32, isOutput=True)

    # Partition p owns rows [p*256, (p+1)*256): contiguous chunk per
    # partition, partition stride 128KB.
    a_v = a_ext.rearrange("(p r) d -> p r d", p=P)
    p_v = p_ext.rearrange("(p r) d -> p r d", p=P)
    n_v = n_ext.rearrange("(p r) d -> p r d", p=P)

    with tile.TileContext(nc) as tc:
        with (
            tc.tile_pool(name="ina", bufs=3) as pool_a,
            tc.tile_pool(name="inp", bufs=3) as pool_p,
            tc.tile_pool(name="inn", bufs=3) as pool_n,
            tc.tile_pool(name="q", bufs=2) as pool_q,
            tc.tile_pool(name="z", bufs=2) as pool_z,
            tc.tile_pool(name="fold", bufs=2) as pool_f1,
            tc.tile_pool(name="fold2", bufs=2) as pool_f2,
            tc.tile_pool(name="fold3", bufs=2) as pool_f3,
            tc.tile_pool(name="acc", bufs=1) as pool_acc,
        ):
            # Per-group accumulators [P, 2, G]: row 0 = apn, row 1 = ap.
            acc_g = [
                pool_acc.tile([P, 2 * GROUPS[g]], F32, name=f"acc{g}", tag=f"acc{g}")
                for g in range(N_GROUPS)
            ]
            f_g = [
                pool_acc.tile([P, GROUPS[g]], F32, name=f"f{g}", tag=f"f{g}")
                for g in range(N_GROUPS)
            ]
            tmp_g = [
                pool_acc.tile([P, GROUPS[g]], F32, name=f"tmp{g}", tag=f"tmp{g}")
                for g in range(N_GROUPS)
            ]
            expf_g = [
                pool_acc.tile([P, GROUPS[g]], F32, name=f"expf{g}", tag=f"expf{g}")
                for g in range(N_GROUPS)
            ]
            negm_g = [
                pool_acc.tile([P, 1], F32, name=f"negm{g}", tag=f"negm{g}")
                for g in range(N_GROUPS)
            ]
            ms_g = [
                pool_acc.tile([P, 2], F32, name=f"ms{g}", tag=f"ms{g}")
                for g in range(N_GROUPS)
            ]

            def logsumexp_group(g):
                # f = c1*apn + c2*ap, then max / exp-accum, ship [P,2] out.
                G = GROUPS[g]
                apn = acc_g[g][:, 0:G]
                ap = acc_g[g][:, G : 2 * G]
                nc.vector.tensor_scalar_mul(tmp_g[g][:], ap, c2)
                nc.vector.scalar_tensor_tensor(
                    f_g[g][:], apn, c1, tmp_g[g][:],
                    op0=mybir.AluOpType.mult, op1=mybir.AluOpType.add,
                )
                nc.vector.tensor_reduce(
                    out=ms_g[g][:, 0:1], in_=f_g[g][:],
                    axis=mybir.AxisListType.X, op=mybir.AluOpType.max,
                )
                nc.vector.tensor_scalar_mul(negm_g[g][:], ms_g[g][:, 0:1], -1.0)
                nc.scalar.activation(
                    out=expf_g[g][:], in_=f_g[g][:],
                    func=mybir.ActivationFunctionType.Exp,
                    bias=negm_g[g][:], scale=1.0,
                    accum_out=ms_g[g][:, 1:2],
                )
                nc.sync.dma_start(
                    out=out_ext[:, 2 * g : 2 * g + 2], in_=ms_g[g][:]
                )

            col = 0
            group_done = 0
            for B in CHUNKS:
                ta = pool_a.tile([P, B * D], CDT, tag="ina")
                tp = pool_p.tile([P, B * D], CDT, tag="inp")
                tn = pool_n.tile([P, B * D], CDT, tag="inn")
                # SWDGE (gpsimd) DMA casts f32 -> bf16 in the datapath.
                nc.gpsimd.dma_start(out=ta[:], in_=a_v[:, col : col + B, :])
                nc.gpsimd.dma_start(out=tp[:], in_=p_v[:, col : col + B, :])
                nc.gpsimd.dma_start(out=tn[:], in_=n_v[:, col : col + B, :])

                # q = a+p, then the two products into one combined tile:
                # z[:, :B*D] = q*n (-> apn), z[:, B*D:] = a*p (-> ap).
                tq = pool_q.tile([P, B * D], CDT, tag="q")
                tz = pool_z.tile([P, 2 * B * D], CDT, tag="z")
                nc.vector.tensor_tensor(tq[:], ta[:], tp[:], mybir.AluOpType.add)
                nc.vector.tensor_tensor(
                    tz[:, : B * D], tq[:], tn[:], mybir.AluOpType.mult
                )
                nc.vector.tensor_tensor(
                    tz[:, B * D :], ta[:], tp[:], mybir.AluOpType.mult
                )

                # Triple fold (bf16 TT 2x) then one 1x-rate reduce for both
                # halves; rows land as [2, B] via a strided output view.
                R = 2 * B
                v0 = tz[:].rearrange("p (r d) -> p r d", d=D)
                t1 = pool_f1.tile([P, R * (D // 2)], CDT, tag="fold")
                nc.vector.tensor_tensor(
                    t1[:], v0[:, :, : D // 2], v0[:, :, D // 2 :],
                    mybir.AluOpType.add,
                )
                v1 = t1[:].rearrange("p (r d) -> p r d", d=D // 2)
                t2 = pool_f2.tile([P, R * (D // 4)], CDT, tag="fold2")
                nc.vector.tensor_tensor(
                    t2[:], v1[:, :, : D // 4], v1[:, :, D // 4 :],
                    mybir.AluOpType.add,
                )
                v2 = t2[:].rearrange("p (r d) -> p r d", d=D // 4)
                t3 = pool_f3.tile([P, R * (D // 8)], CDT, tag="fold3")
                nc.vector.tensor_tensor(
                    t3[:], v2[:, :, : D // 8], v2[:, :, D // 8 :],
                    mybir.AluOpType.add,
                )

                g = _group_of(col)
                gcol = col - GROUP_START[g]
                assert gcol + B <= GROUPS[g]
                G = GROUPS[g]
                dst = acc_g[g][:].rearrange("p (h c) -> p h c", h=2)[
                    :, :, gcol : gcol + B
                ]
                nc.vector.tensor_reduce(
                    out=dst,
                    in_=t3[:].rearrange("p (r d) -> p r d", d=D // 8),
                    axis=mybir.AxisListType.X,
                    op=mybir.AluOpType.add,
                )
                col += B
                # close out any fully-accumulated logsumexp group
                while group_done < N_GROUPS and col >= GROUP_START[group_done] + GROUPS[group_done]:
                    logsumexp_group(group_done)
                    group_done += 1
    nc.compile()
    return nc


def kernel(anchor, positive, negative, alpha):
    anchor = np.ascontiguousarray(np.asarray(anchor, dtype=np.float32))
    positive = np.ascontiguousarray(np.asarray(positive, dtype=np.float32))
    negative = np.ascontiguousarray(np.asarray(negative, dtype=np.float32))
    a_rad = 2.0 * np.pi * float(np.asarray(alpha)) / 360.0
    t2 = float(np.tan(a_rad) ** 2)
    c1 = 4.0 * t2
    c2 = -2.0 * (1.0 + t2)

    nc = _build(c1, c2)
    in_maps = []
    for i in range(NCORES):
        sl = slice(i * N_LOCAL, (i + 1) * N_LOCAL)
        in_maps.append(
            {"anchor": anchor[sl], "positive": positive[sl], "negative": negative[sl]}
        )
    res = run_bass_kernel_spmd(nc, in_maps, core_ids=list(range(NCORES)))

    ms = np.concatenate([np.asarray(r["out"]) for r in res.results], axis=0)
    m = ms[:, 0::2].reshape(-1).astype(np.float64)
    s = ms[:, 1::2].reshape(-1).astype(np.float64)
    M = m.max()
    S = np.sum(s * np.exp(m - M))
    return np.array([np.log(S) + M], dtype=np.float32)


if __name__ == "__main__":
    rng = np.random.default_rng(0)
    out = kernel(
        anchor=rng.standard_normal((N, D), dtype=np.float32),
        positive=rng.standard_normal((N, D), dtype=np.float32),
        negative=rng.standard_normal((N, D), dtype=np.float32),
        alpha=np.int64(45),
    )
    print("kernel out:", out)
